# revision 57
# baseline (speedup 1.0000x reference)
"""Multi-head self-attention kernel for Trainium2 (8 NeuronCores, Bass/Tile).

Sharding: 8 cores = 2 batches x 4 head-groups; each core computes one batch
and 4 heads end to end (no collectives); host sums the 4 bf16 partial
out-projections per batch and folds in the v-bias + out-bias correction.

Per-core dataflow (v2):
  - qkT[ct] = (W.T @ x) + bias, ct in {q01,q23,k01,k24}, [128, S] each
    (two heads stacked on partitions 0:64 / 64:128).
  - scores for (pair p, 512-query window w, key tile j): ONE [128, 1024]
    PSUM tile per j holding both heads side by side; ONE exp per j on ACT.
  - values flipped vs v1: out[q, d] with probs as stationary (N=65 moving
    cols incl a ones column that yields the softmax denominator), PSUM-
    accumulated over all 16 key tiles -> half the tensor-engine rows.
  - normalize on DVE via per-partition reciprocal + tensor_scalar_mul,
    pack two heads into a [128, 128] tile, DMA-XBAR transpose into the
    [d, S] vals layout consumed by the out-projection.
  - out-projection: 16 token tiles, 2x2 accumulated matmuls each, DVE
    copies, DMA out.
Window order: (0,0),(0,1),(1,0),(1,1),(0,2),(0,3),(1,2),(1,3); values of
window i run during window i+3/i+2 (front) or i+1 (back) to keep the PE
fed at the ACT exp pace; out-projections trail once both pairs of a
query window have been transposed.
"""


import numpy as np
import ml_dtypes
import sys

try:
    import concourse.bass as bass
except ImportError:  # pragma: no cover
    sys.path.insert(0, "/opt/trn_rl_repo")
    import concourse.bass as bass

import concourse.bacc as bacc
import concourse.mybir as mybir
import concourse.tile as tile
from concourse.bass_utils import run_bass_kernel_spmd

BF16 = mybir.dt.bfloat16
F32 = mybir.dt.float32
AF = mybir.ActivationFunctionType

D_MODEL = 1024
HEADS_PER_CORE = 4
HEAD_DIM = 64
CH = HEADS_PER_CORE * HEAD_DIM  # 256


def build_core_program(S=2048, D=D_MODEL, reps=1):
    nc = bacc.Bacc(trn_type="TRN2", target_bir_lowering=False, debug=False,
                   enable_partition_id=False)

    xT_d = nc.dram_tensor("xT", [D, S], BF16, kind="ExternalInput").ap()
    wq_d = nc.dram_tensor("wq", [D, CH], BF16, kind="ExternalInput").ap()
    wk_d = nc.dram_tensor("wk", [D, CH], BF16, kind="ExternalInput").ap()
    wv_d = nc.dram_tensor("wv", [D, CH], BF16, kind="ExternalInput").ap()
    wo_d = nc.dram_tensor("wo", [CH, D], BF16, kind="ExternalInput").ap()
    bqk_d = nc.dram_tensor("bqk", [4, 128, 1], F32, kind="ExternalInput").ap()
    out_d = nc.dram_tensor("out", [S, D], BF16, kind="ExternalOutput").ap()

    NT = S // 128     # key tiles
    ND = D // 128     # d_model contraction chunks
    NQ = S // 512     # query windows per pair
    assert NT == 16 and NQ == 4 and ND == 8

    with tile.TileContext(nc) as tc:
        with (
            tc.tile_pool(name="persist", bufs=1) as persist,
            tc.tile_pool(name="probs", bufs=56) as probs_pool,
            tc.tile_pool(name="valsb", bufs=6) as valsb_pool,
            tc.tile_pool(name="rec", bufs=6) as rec_pool,
            tc.tile_pool(name="outb", bufs=2) as outb_pool,
            tc.tile_pool(name="ps_sc", bufs=2, space="PSUM") as ps_sc,
            tc.tile_pool(name="ps_mm", bufs=2, space="PSUM") as ps_mm,
            tc.tile_pool(name="ps_val", bufs=2, space="PSUM") as ps_val,
        ):
            # --- constants ------------------------------------------------
            # consolidated SBUF images: one strided DMA each instead of
            # per-chunk DMAs (HWDGE trigger overhead dominates small DMAs)
            xT_all = persist.tile([128, ND * S], BF16, name="xT", tag="xT")
            wq_all = persist.tile([128, ND * CH], BF16, name="wq", tag="wq")
            wk_all = persist.tile([128, ND * CH], BF16, name="wk", tag="wk")
            wv_all = persist.tile([128, ND * CH], BF16, name="wv", tag="wv")
            bias_all = persist.tile([128, 4], F32, name="bias", tag="bias")
            wo_all = persist.tile([128, 2 * D], BF16, name="wo", tag="wo")

            def xTs(dc, c0, c1):
                return xT_all[:, S * dc + c0:S * dc + c1]

            xT_dr = xT_d.rearrange("(dc p) s -> p dc s", p=128)
            xT_sb = xT_all.rearrange("p (dc s) -> p dc s", s=S)
            # priority order: pieces needed by the first qk chains first;
            # all triggers on SP so the ACT sequencer stays free for exps
            wq_sb = wq_all.rearrange("p (dc ch) -> p dc ch", ch=CH)
            wq_dr = wq_d.rearrange("(dc p) ch -> p dc ch", p=128)
            wk_sb = wk_all.rearrange("p (dc ch) -> p dc ch", ch=CH)
            wk_dr = wk_d.rearrange("(dc p) ch -> p dc ch", p=128)
            # critical path to the first exp: wq -> xT cols 0:512 -> wk;
            # the first k chain is narrowed to key tokens 0:128 so it only
            # trails wk by ~0.5us
            nc.sync.dma_start(
                bias_all.rearrange("p b -> p b ()"),
                bqk_d.rearrange("b p one -> p b one"))
            nc.sync.dma_start(wq_sb, wq_dr)
            nc.sync.dma_start(xT_sb[:, 0:4, 0:512], xT_dr[:, 0:4, 0:512])
            nc.sync.dma_start(xT_sb[:, 4:8, 0:512], xT_dr[:, 4:8, 0:512])
            nc.sync.dma_start(wk_sb, wk_dr)
            nc.sync.dma_start(xT_sb[:, :, 512:1024], xT_dr[:, :, 512:1024])
            nc.sync.dma_start(
                wv_all.rearrange("p (dc ch) -> p dc ch", ch=CH),
                wv_d.rearrange("(dc p) ch -> p dc ch", p=128))
            nc.sync.dma_start(xT_sb[:, :, 1024:1536], xT_dr[:, :, 1024:1536])
            nc.sync.dma_start(xT_sb[:, :, 1536:2048], xT_dr[:, :, 1536:2048])
            nc.sync.dma_start(
                wo_all.rearrange("p (c d) -> p c d", d=D),
                wo_d.rearrange("(c p) d -> p c d", p=128))

            # dependency-free ACT warmup (loads the exp table early)
            warm = persist.tile([128, 1], F32, name="warm", tag="warm")
            nc.vector.memset(warm, 0.0)
            nc.scalar.activation(warm, warm, AF.Exp, bias=0.0, scale=1.0)

            # dependency-free PE warmup: fills the input-DMA head time and
            # establishes the >3us continuous-busy ramp so the first real
            # chains run at the full 2.4 GHz p-state
            pe0 = persist.tile([128, 512], BF16, name="pe0", tag="pe0")
            nc.vector.memset(pe0, 0.0)
            ps_w = ps_mm.tile([128, 512], F32, name="ps_warm", tag="ps_mm")
            for _ in range(6):
                nc.tensor.matmul(ps_w, lhsT=pe0[:, 0:128], rhs=pe0,
                                 start=True, stop=True)

            # bf16 identity for tensor-engine transposes in the endgame
            ident = persist.tile([128, 128], BF16, name="ident", tag="ident")
            nc.vector.memset(ident, 1.0)
            nc.gpsimd.affine_select(ident, ident, [[1, 128]],
                                    mybir.AluOpType.is_equal, 0.0,
                                    base=0, channel_multiplier=-1)

            qkT = [persist.tile([128, S], BF16, name=f"qkT{i}", tag=f"qkT{i}")
                   for i in range(4)]
            # token-major v, 4 heads x (64 dims + ones column)
            vsb = [persist.tile([128, HEADS_PER_CORE * 65], BF16,
                                name=f"v{i}", tag=f"v{i}") for i in range(NT)]
            # [d, S] layout consumed by out-proj (2 heads stacked per pair)
            vals = [persist.tile([128, S], BF16, name=f"vals{i}",
                                 tag=f"vals{i}") for i in range(2)]

            # --- helpers --------------------------------------------------
            qk_ps = {}

            def qk_part(ct, c, half):
                """Half (4 d-chunks) of a qk chain; bias-add on completion."""
                wsrc = wq_all if ct < 2 else wk_all
                wcol = (ct % 2) * 128
                if half == 0:
                    qk_ps[(ct, c)] = ps_mm.tile([128, 512], F32,
                                                name="ps_qk", tag="ps_mm")
                ps = qk_ps[(ct, c)]
                for dc in range(4 * half, 4 * half + 4):
                    nc.tensor.matmul(
                        ps,
                        lhsT=wsrc[:, CH * dc + wcol:CH * dc + wcol + 128],
                        rhs=xTs(dc, 512 * c, 512 * (c + 1)),
                        start=(dc == 0),
                        stop=(dc == ND - 1),
                    )
                if half == 1:
                    nc.vector.tensor_scalar_add(
                        qkT[ct][:, 512 * c:512 * (c + 1)], ps,
                        bias_all[:, ct:ct + 1])

            def qk_chain(ct, c):
                """qkT[ct][:, 512c:512c+512] = (W.T @ x) + bias."""
                qk_part(ct, c, 0)
                qk_part(ct, c, 1)

            def qk_narrow(ct, c0, c1):
                """qkT[ct][:, c0:c1] only — for the latency-critical head."""
                wsrc = wq_all if ct < 2 else wk_all
                wcol = (ct % 2) * 128
                ps = ps_mm.tile([128, c1 - c0], F32, name="ps_qkn",
                                tag="ps_mm")
                for dc in range(ND):
                    nc.tensor.matmul(
                        ps,
                        lhsT=wsrc[:, CH * dc + wcol:CH * dc + wcol + 128],
                        rhs=xTs(dc, c0, c1),
                        start=(dc == 0),
                        stop=(dc == ND - 1),
                    )
                nc.vector.tensor_scalar_add(
                    qkT[ct][:, c0:c1], ps, bias_all[:, ct:ct + 1])

            def v_chain(t):
                """vsb[t]: token-major v for key tile t, ones col per head."""
                ps = ps_mm.tile([128, CH], F32, name="ps_v", tag="ps_mm")
                for dc in range(ND):
                    nc.tensor.matmul(
                        ps,
                        lhsT=xTs(dc, 128 * t, 128 * (t + 1)),
                        rhs=wv_all[:, CH * dc:CH * (dc + 1)],
                        start=(dc == 0),
                        stop=(dc == ND - 1),
                    )
                nc.vector.memset(vsb[t], 1.0)
                nc.vector.tensor_copy(
                    vsb[t].rearrange("p (h c) -> p h c", c=65)[:, :, 0:64],
                    ps.rearrange("p (h c) -> p h c", c=64),
                )

            def scores_exp(p, w, j):
                """One [128, 1024] psum tile: heads 2p|2p+1 scores for key
                tile j x query window w; one exp -> bf16 probs tile."""
                ps = ps_sc.tile([128, 1024], F32, name="ps_sc", tag="ps_sc")
                for hh in range(2):
                    nc.tensor.matmul(
                        ps[:, 512 * hh:512 * (hh + 1)],
                        lhsT=qkT[2 + p][64 * hh:64 * (hh + 1),
                                        128 * j:128 * (j + 1)],
                        rhs=qkT[p][64 * hh:64 * (hh + 1),
                                   512 * w:512 * (w + 1)],
                        start=True, stop=True,
                    )
                pr = probs_pool.tile([128, 1024], BF16, name="probs",
                                     tag="probs")
                nc.scalar.activation(pr, ps, AF.Exp, bias=0.0, scale=0.125)
                return pr

            probs_store = {}
            valsb_store = {}

            def val_mms(ps, p, hh, tq, probs_key, js, first, last):
                """Accumulate probs.T @ v over key tiles `js` into psum."""
                h = 2 * p + hh
                for j in js:
                    pr = probs_store[(probs_key, j)]
                    nc.tensor.matmul(
                        ps,
                        lhsT=pr[:, 512 * hh + 128 * tq:512 * hh + 128 * (tq + 1)],
                        rhs=vsb[j][:, 65 * h:65 * (h + 1)],
                        start=(first and j == js[0]),
                        stop=(last and j == js[-1]),
                        skip_group_check=not (first and last),
                    )

            pe_t_store = {}

            def val_drain(ps, p, w, hh, tq, on_act=False, pe_transpose=False):
                """Normalize psum -> valsb bf16; transpose when pair done."""
                if hh == 0:
                    vb = valsb_pool.tile([128, 128], BF16, name="valsb",
                                         tag="valsb")
                    valsb_store[(p, w, tq)] = vb
                else:
                    vb = valsb_store[(p, w, tq)]
                rc = rec_pool.tile([128, 1], F32, name="rec", tag="rec")
                nc.vector.reciprocal(rc, ps[:, 64:65])
                if on_act:
                    # ACT is idle post-exp: Copy with per-partition scale AP
                    nc.scalar.activation(vb[:, 64 * hh:64 * (hh + 1)],
                                         ps[:, 0:64], AF.Copy, bias=0.0,
                                         scale=rc)
                else:
                    nc.vector.tensor_scalar_mul(
                        vb[:, 64 * hh:64 * (hh + 1)], ps[:, 0:64], rc)
                if hh == 1:
                    if pe_transpose:
                        pe_t_store[tq] = (vb, p, w)
                    else:
                        # both heads packed: [128q, 128d] -> vals[p] via XBAR
                        nc.sync.dma_start(
                            vals[p][:, 512 * w + 128 * tq:
                                    512 * w + 128 * (tq + 1)],
                            vb, transpose=True)

            def pe_transpose_flush(tq, on_act=False):
                """Tensor-engine transpose (latency-critical endgame path)."""
                vb, p, w = pe_t_store.pop(tq)
                psT = ps_sc.tile([128, 128], BF16, name="ps_T", tag="ps_sc")
                nc.tensor.transpose(psT, vb, ident)
                dst = vals[p][:, 512 * w + 128 * tq:512 * w + 128 * (tq + 1)]
                if on_act:
                    nc.scalar.activation(dst, psT, AF.Copy, bias=0.0,
                                         scale=1.0)
                else:
                    nc.vector.tensor_copy(dst, psT)

            def val_chain(p, w, hh, tq, probs_key):
                """values[q, d] for head 2p+hh, q-tile tq of window w."""
                ps = ps_val.tile([128, 65], F32, name="ps_val", tag="ps_val")
                val_mms(ps, p, hh, tq, probs_key, list(range(NT)), True, True)
                val_drain(ps, p, w, hh, tq)

            outb_store = {}

            def outproj_half(t, mh, copy_act=False, pool=None, dma_sp=False):
                """One 512-col half of out tile t; DMA (via the idle Pool
                SWDGE so the SP queue stays clear for transposes) when both
                halves are done."""
                if mh == 0:
                    ob = outb_pool.tile([128, D], BF16, name="outb",
                                        tag="outb")
                    outb_store[t] = ob
                else:
                    ob = outb_store[t]
                if pool is None:
                    ps = ps_mm.tile([128, 512], F32, name="ps_out",
                                    tag="ps_mm")
                else:
                    ps = pool.tile([128, 512], F32, name="ps_out2",
                                   tag="ps_sc")
                for p in range(2):
                    nc.tensor.matmul(
                        ps,
                        lhsT=vals[p][:, 128 * t:128 * (t + 1)],
                        rhs=wo_all[:, D * p + 512 * mh:D * p + 512 * (mh + 1)],
                        start=(p == 0),
                        stop=(p == 1),
                    )
                if copy_act:
                    nc.scalar.activation(ob[:, 512 * mh:512 * (mh + 1)],
                                         ps, AF.Copy, bias=0.0, scale=1.0)
                else:
                    nc.vector.tensor_copy(ob[:, 512 * mh:512 * (mh + 1)], ps)
                if mh == 1:
                    if dma_sp:
                        nc.sync.dma_start(out_d[128 * t:128 * (t + 1), :], ob)
                    else:
                        nc.gpsimd.dma_start(out_d[128 * t:128 * (t + 1), :],
                                            ob)

            # --- schedule -------------------------------------------------
            windows = [(0, 0), (0, 1), (1, 0), (1, 1),
                       (0, 2), (0, 3), (1, 2), (1, 3)]
            # extra chain work per (window idx, j); qk chains split into two
            # 4-chunk halves on adjacent js so per-j PE load stays under the
            # ACT exp pace
            def _qk2(wi, j, ct, c):
                return {(wi, j): ("qkh", ct, c, 0), (wi, j + 1): ("qkh", ct, c, 1)}

            CHAINS = {
                (0, 4): ("v", 0), (0, 5): ("v", 1),
                (0, 8): ("v", 2), (0, 14): ("v", 3),
                (1, 0): ("v", 4), (1, 1): ("v", 5),
                (1, 8): ("v", 6), (1, 9): ("v", 7), (1, 10): ("v", 8),
                (1, 11): ("v", 9),
                (2, 2): ("v", 10), (2, 3): ("v", 11), (2, 6): ("v", 12),
                (2, 7): ("v", 13), (2, 10): ("v", 14), (2, 11): ("v", 15),
            }
            CHAINS.update(_qk2(0, 2, 2, 1))
            CHAINS.update(_qk2(0, 6, 2, 2))
            CHAINS.update(_qk2(0, 10, 2, 3))
            CHAINS.update(_qk2(0, 12, 0, 1))
            CHAINS.update(_qk2(1, 2, 1, 0))
            CHAINS.update(_qk2(1, 4, 3, 0))
            CHAINS.update(_qk2(1, 6, 3, 1))
            CHAINS.update(_qk2(2, 0, 3, 2))
            CHAINS.update(_qk2(2, 4, 3, 3))
            CHAINS.update(_qk2(2, 8, 1, 1))
            CHAINS.update(_qk2(3, 10, 0, 2))
            CHAINS.update(_qk2(4, 2, 0, 3))
            CHAINS.update(_qk2(5, 0, 1, 2))
            CHAINS.update(_qk2(6, 0, 1, 3))
            # values of window FRONT_VALUES[wi] run at js 0-7 of window wi
            FRONT_VALUES = {3: 0, 4: 1, 5: 4, 6: 5, 7: 6}
            # values of window BACK_VALUES[wi] run at js 8-15 of window wi
            BACK_VALUES = {3: 2, 4: 3}
            # out-proj token tiles per (window idx, j)
            # (window, j) -> (token tile, mh half); one half per j so the
            # per-j PE load stays under the ACT exp pace
            OUTPROJ = {}
            for _wi, _base in ((5, 0), (6, 4)):
                for _k in range(8):
                    OUTPROJ[(_wi, 8 + _k)] = (_base + _k // 2, _k % 2)
            OUTPROJ[(7, 10)] = (8, 0)
            OUTPROJ[(7, 11)] = (8, 1)
            OUTPROJ[(7, 13)] = (9, 0)
            OUTPROJ[(7, 14)] = (9, 1)

            def values_step(wi_src, jslot):
                """Chain #jslot (of 8) of window wi_src's values."""
                p, w = windows[wi_src]
                tq, hh = divmod(jslot, 2)
                val_chain(p, w, hh, tq, wi_src)

            # Last window's 8 values chains accumulate into paused psum
            # groups (4 chains packed per bank; only the first opens the
            # group, zeroing the whole bank's zero-region): js 0-6 in one
            # batch at j=8 (once ps_val is free of the front values), then
            # one key tile per j; the j=14,15 matmuls + drains run after the
            # final exp so only ~2 matmuls/chain trail the last score.
            def tail_part(tail_ps, jslot, js, first, last):
                bank, k = divmod(jslot, 4)
                ps = tail_ps[bank][:, 65 * k:65 * (k + 1)]
                tq, hh = divmod(jslot, 2)
                val_mms(ps, 1, hh, tq, 7, js, first, last)

            for _rep in range(reps):
                qk_chain(0, 0)
                qk_narrow(2, 0, 128)
                qk_narrow(2, 128, 512)
                tail_ps = {}
                for wi, (p, w) in enumerate(windows):
                    for j in range(NT):
                        extra = CHAINS.get((wi, j))
                        if extra is not None:
                            if extra[0] == "qkh":
                                qk_part(extra[1], extra[2], extra[3])
                            else:
                                v_chain(extra[1])
                        if wi in FRONT_VALUES and j < 8:
                            values_step(FRONT_VALUES[wi], j)
                        if wi in BACK_VALUES and j >= 8:
                            values_step(BACK_VALUES[wi], j - 8)
                        probs_store[(wi, j)] = scores_exp(p, w, j)
                        oph = OUTPROJ.get((wi, j))
                        if oph is not None:
                            outproj_half(*oph)
                        if wi == 7 and j in (8, 9):
                            bank = j - 8
                            tail_ps[bank] = ps_val.tile(
                                [128, 260], F32, name="ps_tail",
                                tag="ps_val")
                            for jslot in range(4 * bank, 4 * bank + 4):
                                tail_part(tail_ps, jslot, list(range(7)),
                                          jslot % 4 == 0, False)
                        elif wi == 7 and j == 10:
                            for jslot in range(8):
                                tail_part(tail_ps, jslot, [7, 8], False,
                                          False)
                        elif wi == 7 and j >= 11:
                            for jslot in range(8):
                                tail_part(tail_ps, jslot, [j - 2], False,
                                          False)
                            if j == 15:
                                # j=14 probs are exp'd by now; only the j=15
                                # matmuls remain for after the final exp
                                for jslot in range(8):
                                    tail_part(tail_ps, jslot, [14], False,
                                              False)
                # tail: last two key tiles of each paused chain; drains split
                # ACT/DVE; transposes on the tensor engine (no DMA latency),
                # out-proj 10/11 matmuls fill the PE between them
                for jslot in range(7, -1, -1):
                    tail_part(tail_ps, jslot, [15], False, True)
                for tq in range(3, -1, -1):
                    for hh in range(2):
                        bank, k = divmod(2 * tq + hh, 4)
                        val_drain(tail_ps[bank][:, 65 * k:65 * (k + 1)],
                                  1, 3, hh, tq, on_act=(hh == 0),
                                  pe_transpose=True)
                outproj_half(10, 0)
                pe_transpose_flush(3)
                outproj_half(10, 1)
                pe_transpose_flush(2, on_act=True)
                outproj_half(11, 0, copy_act=True)
                pe_transpose_flush(1)
                outproj_half(11, 1, copy_act=True)
                pe_transpose_flush(0, on_act=True)
                for t in (15, 14, 13, 12):
                    pool = ps_sc if t in (15, 13) else None
                    # alternate copy engines so the two halves land in
                    # parallel on ACT and DVE; spread the final DMAs over
                    # the SP and Pool queues so they don't serialize
                    outproj_half(t, 0, copy_act=True, pool=pool, dma_sp=True)
                    outproj_half(t, 1, copy_act=False, pool=pool,
                                 dma_sp=True)
                # drop references so the next rep re-allocates cleanly
                probs_store.clear()
                valsb_store.clear()
                # drop references so the next rep re-allocates cleanly
                probs_store.clear()
                valsb_store.clear()

    nc.compile()
    return nc


def make_in_maps(x, W_qkv, b_qkv, W_out, n_cores=8):
    """Per-core input dict: core c -> batch c//4, head group c%4."""
    bf = ml_dtypes.bfloat16
    in_maps = []
    for c in range(n_cores):
        b, g = divmod(c, 4)
        heads = range(HEADS_PER_CORE * g, HEADS_PER_CORE * (g + 1))
        qs = np.concatenate([W_qkv[:, 192 * h:192 * h + 64] for h in heads], 1)
        ks = np.concatenate([W_qkv[:, 192 * h + 64:192 * h + 128] for h in heads], 1)
        vs = np.concatenate([W_qkv[:, 192 * h + 128:192 * h + 192] for h in heads], 1)
        bq = np.concatenate([b_qkv[192 * h:192 * h + 64] for h in heads])
        bk = np.concatenate([b_qkv[192 * h + 64:192 * h + 128] for h in heads])
        in_maps.append({
            "xT": np.ascontiguousarray(x[b].T).astype(bf),
            "wq": np.ascontiguousarray(qs).astype(bf),
            "wk": np.ascontiguousarray(ks).astype(bf),
            "wv": np.ascontiguousarray(vs).astype(bf),
            "wo": np.ascontiguousarray(W_out[CH * g:CH * (g + 1)]).astype(bf),
            "bqk": np.stack([bq[:128], bq[128:], bk[:128], bk[128:]])
                     .reshape(4, 128, 1).astype(np.float32),
        })
    return in_maps


_PROGRAM_CACHE = {}


def _get_program(S):
    if S not in _PROGRAM_CACHE:
        _PROGRAM_CACHE[S] = build_core_program(S=S)
    return _PROGRAM_CACHE[S]


class PjrtRunner:
    """Reusable compiled SPMD executable (no donation, so it can be re-run
    back-to-back on device-resident inputs for timing)."""

    def __init__(self, nc, n_cores=8):
        import jax
        from jax.sharding import Mesh, PartitionSpec
        from jax.experimental.shard_map import shard_map
        from concourse import bass2jax, mybir as mb

        bass2jax.install_neuronx_cc_hook()
        self.nc = nc
        self.n_cores = n_cores
        in_names, out_names, out_avals, zero_outs = [], [], [], []
        for alloc in nc.m.functions[0].allocations:
            if not isinstance(alloc, mb.MemoryLocationSet):
                continue
            name = alloc.memorylocations[0].name
            if alloc.kind == "ExternalInput":
                in_names.append(name)
            elif alloc.kind == "ExternalOutput":
                out_names.append(name)
                shape = tuple(alloc.tensor_shape)
                dtype = mb.dt.np(alloc.dtype)
                out_avals.append(jax.core.ShapedArray(shape, dtype))
                zero_outs.append(np.zeros(shape, dtype))
        self.in_names = list(in_names)
        self.out_names = out_names
        self.out_avals = out_avals
        self.zero_outs = zero_outs
        n_params = len(in_names)
        all_names = in_names + out_names

        def _body(*args):
            outs = bass2jax._bass_exec_p.bind(
                *args,
                out_avals=tuple(out_avals),
                in_names=tuple(all_names),
                out_names=tuple(out_names),
                lowering_input_output_aliases=(),
                sim_require_finite=True,
                sim_require_nnan=True,
                nc=nc,
            )
            return tuple(outs)

        devices = jax.devices()[:n_cores]
        self.mesh = Mesh(np.asarray(devices), ("core",))
        in_specs = (PartitionSpec("core"),) * (n_params + len(out_names))
        out_specs = (PartitionSpec("core"),) * len(out_names)
        self.fn = jax.jit(
            shard_map(_body, mesh=self.mesh, in_specs=in_specs,
                      out_specs=out_specs, check_rep=False),
            keep_unused=True,
        )
        self._dev_args = None

    def stage(self, in_maps):
        """Concatenate per-core inputs, upload once, keep device arrays."""
        import jax
        from jax.sharding import NamedSharding, PartitionSpec
        n = self.n_cores
        concat = [
            np.concatenate([np.asarray(in_maps[c][k]) for c in range(n)], axis=0)
            for k in self.in_names
        ]
        concat += [
            np.zeros((n * z.shape[0], *z.shape[1:]), z.dtype)
            for z in self.zero_outs
        ]
        sh = NamedSharding(self.mesh, PartitionSpec("core"))
        self._dev_args = [jax.device_put(a, sh) for a in concat]

    def run(self):
        outs = self.fn(*self._dev_args)
        # keep device arrays for reuse; pull results to host
        res = []
        for c in range(self.n_cores):
            res.append({
                name: np.asarray(outs[i]).reshape(
                    self.n_cores, *self.out_avals[i].shape)[c]
                for i, name in enumerate(self.out_names)
            })
        return res

    def time_iters(self, iters=20):
        import time
        import jax
        outs = self.fn(*self._dev_args)
        jax.block_until_ready(outs)
        t0 = time.perf_counter()
        for _ in range(iters):
            outs = self.fn(*self._dev_args)
        jax.block_until_ready(outs)
        t1 = time.perf_counter()
        return (t1 - t0) / iters


_RUNNER_CACHE = {}


def get_runner(S):
    if S not in _RUNNER_CACHE:
        _RUNNER_CACHE[S] = PjrtRunner(_get_program(S))
    return _RUNNER_CACHE[S]


def combine_outputs(results, W_qkv, b_qkv, W_out, b_out, B, S, D):
    b_v = np.concatenate([b_qkv[192 * h + 128:192 * h + 192] for h in range(16)])
    corr = (b_v.astype(np.float64) @ W_out.astype(np.float64)).astype(np.float32)
    corr += b_out
    out = np.zeros((B, S, D), np.float32)
    for c in range(8):
        out[c // 4] += results[c]["out"].astype(np.float32)
    out += corr[None, None, :]
    return out


def kernel(x, W_qkv, b_qkv, W_out, b_out):
    x = np.asarray(x)
    W_qkv = np.asarray(W_qkv)
    b_qkv = np.asarray(b_qkv)
    W_out = np.asarray(W_out)
    b_out = np.asarray(b_out)
    B, S, D = x.shape

    runner = get_runner(S)
    runner.stage(make_in_maps(x, W_qkv, b_qkv, W_out))
    results = runner.run()
    return combine_outputs(results, W_qkv, b_qkv, W_out, b_out, B, S, D)


# revision 60
# speedup vs baseline: 1.0668x; 1.0668x over previous
"""Multi-head self-attention kernel for Trainium2 (8 NeuronCores, Bass/Tile).

Sharding: 8 cores = 2 batches x 4 head-groups; each core computes one batch
and 4 heads end to end (no collectives); host sums the 4 bf16 partial
out-projections per batch and folds in the v-bias + out-bias correction.

Per-core dataflow (v2):
  - qkT[ct] = (W.T @ x) + bias, ct in {q01,q23,k01,k24}, [128, S] each
    (two heads stacked on partitions 0:64 / 64:128).
  - scores for (pair p, 512-query window w, key tile j): ONE [128, 1024]
    PSUM tile per j holding both heads side by side; ONE exp per j on ACT.
  - values flipped vs v1: out[q, d] with probs as stationary (N=65 moving
    cols incl a ones column that yields the softmax denominator), PSUM-
    accumulated over all 16 key tiles -> half the tensor-engine rows.
  - normalize on DVE via per-partition reciprocal + tensor_scalar_mul,
    pack two heads into a [128, 128] tile, DMA-XBAR transpose into the
    [d, S] vals layout consumed by the out-projection.
  - out-projection: 16 token tiles, 2x2 accumulated matmuls each, DVE
    copies, DMA out.
Window order: (0,0),(0,1),(1,0),(1,1),(0,2),(0,3),(1,2),(1,3); values of
window i run during window i+3/i+2 (front) or i+1 (back) to keep the PE
fed at the ACT exp pace; out-projections trail once both pairs of a
query window have been transposed.
"""


import numpy as np
import ml_dtypes
import sys

try:
    import concourse.bass as bass
except ImportError:  # pragma: no cover
    sys.path.insert(0, "/opt/trn_rl_repo")
    import concourse.bass as bass

import concourse.bacc as bacc
import concourse.mybir as mybir
import concourse.tile as tile
from concourse.bass_utils import run_bass_kernel_spmd

BF16 = mybir.dt.bfloat16
F32 = mybir.dt.float32
AF = mybir.ActivationFunctionType

D_MODEL = 1024
HEADS_PER_CORE = 4
HEAD_DIM = 64
CH = HEADS_PER_CORE * HEAD_DIM  # 256


def build_core_program(S=2048, D=D_MODEL, reps=1):
    nc = bacc.Bacc(trn_type="TRN2", target_bir_lowering=False, debug=False,
                   enable_partition_id=False)

    xT_d = nc.dram_tensor("xT", [D, S], BF16, kind="ExternalInput").ap()
    wq_d = nc.dram_tensor("wq", [D, CH], BF16, kind="ExternalInput").ap()
    wk_d = nc.dram_tensor("wk", [D, CH], BF16, kind="ExternalInput").ap()
    wv_d = nc.dram_tensor("wv", [D, CH], BF16, kind="ExternalInput").ap()
    wo_d = nc.dram_tensor("wo", [CH, D], BF16, kind="ExternalInput").ap()
    bqk_d = nc.dram_tensor("bqk", [4, 128, 1], F32, kind="ExternalInput").ap()
    out_d = nc.dram_tensor("out", [S, D], BF16, kind="ExternalOutput").ap()

    NT = S // 128     # key tiles
    ND = D // 128     # d_model contraction chunks
    NQ = S // 512     # query windows per pair
    assert NT == 16 and NQ == 4 and ND == 8

    with tile.TileContext(nc) as tc:
        with (
            tc.tile_pool(name="persist", bufs=1) as persist,
            tc.tile_pool(name="probs", bufs=56) as probs_pool,
            tc.tile_pool(name="valsb", bufs=6) as valsb_pool,
            tc.tile_pool(name="rec", bufs=6) as rec_pool,
            tc.tile_pool(name="outb", bufs=2) as outb_pool,
            tc.tile_pool(name="ps_sc", bufs=2, space="PSUM") as ps_sc,
            tc.tile_pool(name="ps_mm", bufs=2, space="PSUM") as ps_mm,
            tc.tile_pool(name="ps_val", bufs=2, space="PSUM") as ps_val,
        ):
            # --- constants ------------------------------------------------
            # consolidated SBUF images: one strided DMA each instead of
            # per-chunk DMAs (HWDGE trigger overhead dominates small DMAs)
            xT_all = persist.tile([128, ND * S], BF16, name="xT", tag="xT")
            wq_all = persist.tile([128, ND * CH], BF16, name="wq", tag="wq")
            wk_all = persist.tile([128, ND * CH], BF16, name="wk", tag="wk")
            wv_all = persist.tile([128, ND * CH], BF16, name="wv", tag="wv")
            bias_all = persist.tile([128, 4], F32, name="bias", tag="bias")
            wo_all = persist.tile([128, 2 * D], BF16, name="wo", tag="wo")

            def xTs(dc, c0, c1):
                return xT_all[:, S * dc + c0:S * dc + c1]

            xT_dr = xT_d.rearrange("(dc p) s -> p dc s", p=128)
            xT_sb = xT_all.rearrange("p (dc s) -> p dc s", s=S)
            # priority order: pieces needed by the first qk chains first;
            # all triggers on SP so the ACT sequencer stays free for exps
            wq_sb = wq_all.rearrange("p (dc ch) -> p dc ch", ch=CH)
            wq_dr = wq_d.rearrange("(dc p) ch -> p dc ch", p=128)
            wk_sb = wk_all.rearrange("p (dc ch) -> p dc ch", ch=CH)
            wk_dr = wk_d.rearrange("(dc p) ch -> p dc ch", p=128)
            # critical path to the first exp: wq -> xT cols 0:512 -> wk;
            # the first k chain is narrowed to key tokens 0:128 so it only
            # trails wk by ~0.5us
            nc.sync.dma_start(
                bias_all.rearrange("p b -> p b ()"),
                bqk_d.rearrange("b p one -> p b one"))
            nc.sync.dma_start(wq_sb, wq_dr)
            nc.sync.dma_start(xT_sb[:, 0:4, 0:512], xT_dr[:, 0:4, 0:512])
            nc.sync.dma_start(xT_sb[:, 4:8, 0:512], xT_dr[:, 4:8, 0:512])
            nc.sync.dma_start(wk_sb, wk_dr)
            nc.sync.dma_start(xT_sb[:, :, 512:1024], xT_dr[:, :, 512:1024])
            nc.sync.dma_start(
                wv_all.rearrange("p (dc ch) -> p dc ch", ch=CH),
                wv_d.rearrange("(dc p) ch -> p dc ch", p=128))
            nc.sync.dma_start(xT_sb[:, :, 1024:1536], xT_dr[:, :, 1024:1536])
            nc.sync.dma_start(xT_sb[:, :, 1536:2048], xT_dr[:, :, 1536:2048])
            nc.sync.dma_start(
                wo_all.rearrange("p (c d) -> p c d", d=D),
                wo_d.rearrange("(c p) d -> p c d", p=128))

            # dependency-free ACT warmup (loads the exp table early)
            warm = persist.tile([128, 1], F32, name="warm", tag="warm")
            nc.vector.memset(warm, 0.0)
            nc.scalar.activation(warm, warm, AF.Exp, bias=0.0, scale=1.0)

            # dependency-free PE warmup: fills the input-DMA head time and
            # establishes the >3us continuous-busy ramp so the first real
            # chains run at the full 2.4 GHz p-state
            pe0 = persist.tile([128, 512], BF16, name="pe0", tag="pe0")
            nc.vector.memset(pe0, 0.0)
            ps_w = ps_mm.tile([128, 512], F32, name="ps_warm", tag="ps_mm")
            for _ in range(6):
                nc.tensor.matmul(ps_w, lhsT=pe0[:, 0:128], rhs=pe0,
                                 start=True, stop=True)

            # bf16 identity for tensor-engine transposes in the endgame
            ident = persist.tile([128, 128], BF16, name="ident", tag="ident")
            nc.vector.memset(ident, 1.0)
            nc.gpsimd.affine_select(ident, ident, [[1, 128]],
                                    mybir.AluOpType.is_equal, 0.0,
                                    base=0, channel_multiplier=-1)

            qkT = [persist.tile([128, S], BF16, name=f"qkT{i}", tag=f"qkT{i}")
                   for i in range(4)]
            # token-major v, 4 heads x (64 dims + ones column)
            vsb = [persist.tile([128, HEADS_PER_CORE * 65], BF16,
                                name=f"v{i}", tag=f"v{i}") for i in range(NT)]
            # [d, S] layout consumed by out-proj (2 heads stacked per pair)
            vals = [persist.tile([128, S], BF16, name=f"vals{i}",
                                 tag=f"vals{i}") for i in range(2)]

            # --- helpers --------------------------------------------------
            qk_ps = {}

            def qk_part(ct, c, half):
                """Half (4 d-chunks) of a qk chain; bias-add on completion."""
                wsrc = wq_all if ct < 2 else wk_all
                wcol = (ct % 2) * 128
                if half == 0:
                    qk_ps[(ct, c)] = ps_mm.tile([128, 512], F32,
                                                name="ps_qk", tag="ps_mm")
                ps = qk_ps[(ct, c)]
                for dc in range(4 * half, 4 * half + 4):
                    nc.tensor.matmul(
                        ps,
                        lhsT=wsrc[:, CH * dc + wcol:CH * dc + wcol + 128],
                        rhs=xTs(dc, 512 * c, 512 * (c + 1)),
                        start=(dc == 0),
                        stop=(dc == ND - 1),
                    )
                if half == 1:
                    nc.vector.tensor_scalar_add(
                        qkT[ct][:, 512 * c:512 * (c + 1)], ps,
                        bias_all[:, ct:ct + 1])

            def qk_chain(ct, c):
                """qkT[ct][:, 512c:512c+512] = (W.T @ x) + bias."""
                qk_part(ct, c, 0)
                qk_part(ct, c, 1)

            def qk_narrow(ct, c0, c1):
                """qkT[ct][:, c0:c1] only — for the latency-critical head."""
                wsrc = wq_all if ct < 2 else wk_all
                wcol = (ct % 2) * 128
                ps = ps_mm.tile([128, c1 - c0], F32, name="ps_qkn",
                                tag="ps_mm")
                for dc in range(ND):
                    nc.tensor.matmul(
                        ps,
                        lhsT=wsrc[:, CH * dc + wcol:CH * dc + wcol + 128],
                        rhs=xTs(dc, c0, c1),
                        start=(dc == 0),
                        stop=(dc == ND - 1),
                    )
                nc.vector.tensor_scalar_add(
                    qkT[ct][:, c0:c1], ps, bias_all[:, ct:ct + 1])

            def v_chain(t):
                """vsb[t]: token-major v for key tile t, ones col per head."""
                ps = ps_mm.tile([128, CH], F32, name="ps_v", tag="ps_mm")
                for dc in range(ND):
                    nc.tensor.matmul(
                        ps,
                        lhsT=xTs(dc, 128 * t, 128 * (t + 1)),
                        rhs=wv_all[:, CH * dc:CH * (dc + 1)],
                        start=(dc == 0),
                        stop=(dc == ND - 1),
                    )
                nc.vector.memset(vsb[t], 1.0)
                nc.vector.tensor_copy(
                    vsb[t].rearrange("p (h c) -> p h c", c=65)[:, :, 0:64],
                    ps.rearrange("p (h c) -> p h c", c=64),
                )

            def scores_exp(p, w, j):
                """One [128, 1024] psum tile: heads 2p|2p+1 scores for key
                tile j x query window w; one exp -> bf16 probs tile."""
                ps = ps_sc.tile([128, 1024], F32, name="ps_sc", tag="ps_sc")
                for hh in range(2):
                    nc.tensor.matmul(
                        ps[:, 512 * hh:512 * (hh + 1)],
                        lhsT=qkT[2 + p][64 * hh:64 * (hh + 1),
                                        128 * j:128 * (j + 1)],
                        rhs=qkT[p][64 * hh:64 * (hh + 1),
                                   512 * w:512 * (w + 1)],
                        start=True, stop=True,
                    )
                pr = probs_pool.tile([128, 1024], BF16, name="probs",
                                     tag="probs")
                nc.scalar.activation(pr, ps, AF.Exp, bias=0.0, scale=0.125)
                return pr

            probs_store = {}
            valsb_store = {}

            def val_mms(ps, p, hh, tq, probs_key, js, first, last):
                """Accumulate probs.T @ v over key tiles `js` into psum."""
                h = 2 * p + hh
                for j in js:
                    pr = probs_store[(probs_key, j)]
                    nc.tensor.matmul(
                        ps,
                        lhsT=pr[:, 512 * hh + 128 * tq:512 * hh + 128 * (tq + 1)],
                        rhs=vsb[j][:, 65 * h:65 * (h + 1)],
                        start=(first and j == js[0]),
                        stop=(last and j == js[-1]),
                        skip_group_check=not (first and last),
                    )

            pe_t_store = {}

            def val_drain(ps, p, w, hh, tq, on_act=False, pe_transpose=False):
                """Normalize psum -> valsb bf16; transpose when pair done."""
                if hh == 0:
                    vb = valsb_pool.tile([128, 128], BF16, name="valsb",
                                         tag="valsb")
                    valsb_store[(p, w, tq)] = vb
                else:
                    vb = valsb_store[(p, w, tq)]
                rc = rec_pool.tile([128, 1], F32, name="rec", tag="rec")
                nc.vector.reciprocal(rc, ps[:, 64:65])
                if on_act:
                    # ACT is idle post-exp: Copy with per-partition scale AP
                    nc.scalar.activation(vb[:, 64 * hh:64 * (hh + 1)],
                                         ps[:, 0:64], AF.Copy, bias=0.0,
                                         scale=rc)
                else:
                    nc.vector.tensor_scalar_mul(
                        vb[:, 64 * hh:64 * (hh + 1)], ps[:, 0:64], rc)
                if hh == 1:
                    if pe_transpose:
                        pe_t_store[tq] = (vb, p, w)
                    else:
                        # both heads packed: [128q, 128d] -> vals[p] via XBAR
                        nc.sync.dma_start(
                            vals[p][:, 512 * w + 128 * tq:
                                    512 * w + 128 * (tq + 1)],
                            vb, transpose=True)

            def pe_transpose_flush(tq, on_act=False):
                """Tensor-engine transpose (latency-critical endgame path)."""
                vb, p, w = pe_t_store.pop(tq)
                psT = ps_sc.tile([128, 128], BF16, name="ps_T", tag="ps_sc")
                nc.tensor.transpose(psT, vb, ident)
                dst = vals[p][:, 512 * w + 128 * tq:512 * w + 128 * (tq + 1)]
                if on_act:
                    nc.scalar.activation(dst, psT, AF.Copy, bias=0.0,
                                         scale=1.0)
                else:
                    nc.vector.tensor_copy(dst, psT)

            def val_chain(p, w, hh, tq, probs_key):
                """values[q, d] for head 2p+hh, q-tile tq of window w."""
                ps = ps_val.tile([128, 65], F32, name="ps_val", tag="ps_val")
                val_mms(ps, p, hh, tq, probs_key, list(range(NT)), True, True)
                val_drain(ps, p, w, hh, tq)

            outb_store = {}

            def outproj_half(t, mh, copy_act=False, pool=None, dma_sp=False):
                """One 512-col half of out tile t; DMA (via the idle Pool
                SWDGE so the SP queue stays clear for transposes) when both
                halves are done."""
                if mh == 0:
                    ob = outb_pool.tile([128, D], BF16, name="outb",
                                        tag="outb")
                    outb_store[t] = ob
                else:
                    ob = outb_store[t]
                if pool is None:
                    ps = ps_mm.tile([128, 512], F32, name="ps_out",
                                    tag="ps_mm")
                else:
                    ps = pool.tile([128, 512], F32, name="ps_out2",
                                   tag="ps_sc")
                for p in range(2):
                    nc.tensor.matmul(
                        ps,
                        lhsT=vals[p][:, 128 * t:128 * (t + 1)],
                        rhs=wo_all[:, D * p + 512 * mh:D * p + 512 * (mh + 1)],
                        start=(p == 0),
                        stop=(p == 1),
                    )
                if copy_act:
                    nc.scalar.activation(ob[:, 512 * mh:512 * (mh + 1)],
                                         ps, AF.Copy, bias=0.0, scale=1.0)
                else:
                    nc.vector.tensor_copy(ob[:, 512 * mh:512 * (mh + 1)], ps)
                if mh == 1:
                    if dma_sp:
                        nc.sync.dma_start(out_d[128 * t:128 * (t + 1), :], ob)
                    else:
                        nc.gpsimd.dma_start(out_d[128 * t:128 * (t + 1), :],
                                            ob)

            # --- schedule -------------------------------------------------
            windows = [(0, 0), (0, 1), (1, 0), (1, 1),
                       (0, 2), (0, 3), (1, 2), (1, 3)]
            # extra chain work per (window idx, j); qk chains split into two
            # 4-chunk halves on adjacent js so per-j PE load stays under the
            # ACT exp pace
            def _qk2(wi, j, ct, c):
                return {(wi, j): ("qkh", ct, c, 0), (wi, j + 1): ("qkh", ct, c, 1)}

            CHAINS = {
                (0, 4): ("v", 0), (0, 5): ("v", 1),
                (0, 8): ("v", 2), (0, 14): ("v", 3),
                (1, 0): ("v", 4), (1, 1): ("v", 5),
                (1, 8): ("v", 6), (1, 9): ("v", 7), (1, 10): ("v", 8),
                (1, 11): ("v", 9),
                (2, 2): ("v", 10), (2, 3): ("v", 11), (2, 6): ("v", 12),
                (2, 7): ("v", 13), (2, 10): ("v", 14), (2, 11): ("v", 15),
            }
            CHAINS.update(_qk2(0, 2, 2, 1))
            CHAINS.update(_qk2(0, 6, 2, 2))
            CHAINS.update(_qk2(0, 10, 2, 3))
            CHAINS.update(_qk2(0, 12, 0, 1))
            CHAINS.update(_qk2(1, 2, 1, 0))
            CHAINS.update(_qk2(1, 4, 3, 0))
            CHAINS.update(_qk2(1, 6, 3, 1))
            CHAINS.update(_qk2(2, 0, 3, 2))
            CHAINS.update(_qk2(2, 4, 3, 3))
            CHAINS.update(_qk2(2, 8, 1, 1))
            CHAINS.update(_qk2(3, 10, 0, 2))
            CHAINS.update(_qk2(4, 2, 0, 3))
            CHAINS.update(_qk2(5, 0, 1, 2))
            CHAINS.update(_qk2(6, 0, 1, 3))
            # values of window FRONT_VALUES[wi] run at js 0-7 of window wi
            FRONT_VALUES = {3: 0, 4: 1, 5: 4, 6: 5, 7: 6}
            # values of window BACK_VALUES[wi] run at js 8-15 of window wi
            BACK_VALUES = {3: 2, 4: 3}
            # out-proj token tiles per (window idx, j)
            # (window, j) -> (token tile, mh half); one half per j so the
            # per-j PE load stays under the ACT exp pace
            OUTPROJ = {}
            for _wi, _base in ((5, 0), (6, 4)):
                for _k in range(8):
                    OUTPROJ[(_wi, 8 + _k)] = (_base + _k // 2, _k % 2)
            OUTPROJ[(7, 10)] = (8, 0)
            OUTPROJ[(7, 11)] = (8, 1)
            OUTPROJ[(7, 13)] = (9, 0)
            OUTPROJ[(7, 14)] = (9, 1)

            def values_step(wi_src, jslot):
                """Chain #jslot (of 8) of window wi_src's values."""
                p, w = windows[wi_src]
                tq, hh = divmod(jslot, 2)
                val_chain(p, w, hh, tq, wi_src)

            # When a window's front values read the IMMEDIATELY previous
            # window's probs, the chain's last matmuls (j=14,15) race the
            # final exps and block the in-order PE right before scores j0.
            # Split: js 0-13 at slot j, paused [14,15] + drain at slot j+1.
            split_state = {}

            def front_step(wi, j):
                src = FRONT_VALUES[wi]
                if src != wi - 1:
                    if j < 8:
                        values_step(src, j)
                    return
                prev = split_state.pop(wi, None)
                if prev is not None:
                    ps, p, w, hh, tq = prev
                    val_mms(ps, p, hh, tq, src, [14, 15], False, True)
                    val_drain(ps, p, w, hh, tq)
                if j < 8:
                    p, w = windows[src]
                    tq, hh = divmod(j, 2)
                    ps = ps_val.tile([128, 65], F32, name="ps_val",
                                     tag="ps_val")
                    val_mms(ps, p, hh, tq, src, list(range(14)), True, False)
                    split_state[wi] = (ps, p, w, hh, tq)

            # Last window's 8 values chains accumulate into paused psum
            # groups (4 chains packed per bank; only the first opens the
            # group, zeroing the whole bank's zero-region): js 0-6 in one
            # batch at j=8 (once ps_val is free of the front values), then
            # one key tile per j; the j=14,15 matmuls + drains run after the
            # final exp so only ~2 matmuls/chain trail the last score.
            def tail_part(tail_ps, jslot, js, first, last):
                bank, k = divmod(jslot, 4)
                ps = tail_ps[bank][:, 65 * k:65 * (k + 1)]
                tq, hh = divmod(jslot, 2)
                val_mms(ps, 1, hh, tq, 7, js, first, last)

            for _rep in range(reps):
                qk_chain(0, 0)
                qk_narrow(2, 0, 128)
                qk_narrow(2, 128, 512)
                tail_ps = {}
                for wi, (p, w) in enumerate(windows):
                    for j in range(NT):
                        extra = CHAINS.get((wi, j))
                        if extra is not None:
                            if extra[0] == "qkh":
                                qk_part(extra[1], extra[2], extra[3])
                            else:
                                v_chain(extra[1])
                        if wi in FRONT_VALUES and j < 8:
                            values_step(FRONT_VALUES[wi], j)
                        if wi in BACK_VALUES and j >= 8:
                            values_step(BACK_VALUES[wi], j - 8)
                        probs_store[(wi, j)] = scores_exp(p, w, j)
                        oph = OUTPROJ.get((wi, j))
                        if oph is not None:
                            outproj_half(*oph)
                        if wi == 7 and j in (8, 9):
                            bank = j - 8
                            tail_ps[bank] = ps_val.tile(
                                [128, 260], F32, name="ps_tail",
                                tag="ps_val")
                            for jslot in range(4 * bank, 4 * bank + 4):
                                tail_part(tail_ps, jslot, list(range(7)),
                                          jslot % 4 == 0, False)
                        elif wi == 7 and j == 10:
                            for jslot in range(8):
                                tail_part(tail_ps, jslot, [7, 8], False,
                                          False)
                        elif wi == 7 and j >= 11:
                            for jslot in range(8):
                                tail_part(tail_ps, jslot, [j - 2], False,
                                          False)
                            if j == 15:
                                # j=14 probs are exp'd by now; only the j=15
                                # matmuls remain for after the final exp
                                for jslot in range(8):
                                    tail_part(tail_ps, jslot, [14], False,
                                              False)
                # tail: last two key tiles of each paused chain; drains split
                # ACT/DVE; transposes on the tensor engine (no DMA latency),
                # out-proj 10/11 matmuls fill the PE between them
                for jslot in range(7, -1, -1):
                    tail_part(tail_ps, jslot, [15], False, True)
                for tq in range(3, -1, -1):
                    for hh in range(2):
                        bank, k = divmod(2 * tq + hh, 4)
                        val_drain(tail_ps[bank][:, 65 * k:65 * (k + 1)],
                                  1, 3, hh, tq, on_act=(hh == 0),
                                  pe_transpose=True)
                outproj_half(10, 0)
                pe_transpose_flush(3)
                outproj_half(10, 1)
                pe_transpose_flush(2, on_act=True)
                outproj_half(11, 0, copy_act=True)
                pe_transpose_flush(1)
                outproj_half(11, 1, copy_act=True)
                pe_transpose_flush(0, on_act=True)
                for t in (15, 14, 13, 12):
                    pool = ps_sc if t in (15, 13) else None
                    # alternate copy engines so the two halves land in
                    # parallel on ACT and DVE; spread the final DMAs over
                    # the SP and Pool queues so they don't serialize
                    outproj_half(t, 0, copy_act=True, pool=pool, dma_sp=True)
                    outproj_half(t, 1, copy_act=False, pool=pool,
                                 dma_sp=True)
                # drop references so the next rep re-allocates cleanly
                probs_store.clear()
                valsb_store.clear()
                # drop references so the next rep re-allocates cleanly
                probs_store.clear()
                valsb_store.clear()

    nc.compile()
    return nc


def make_in_maps(x, W_qkv, b_qkv, W_out, n_cores=8):
    """Per-core input dict: core c -> batch c//4, head group c%4."""
    bf = ml_dtypes.bfloat16
    in_maps = []
    for c in range(n_cores):
        b, g = divmod(c, 4)
        heads = range(HEADS_PER_CORE * g, HEADS_PER_CORE * (g + 1))
        qs = np.concatenate([W_qkv[:, 192 * h:192 * h + 64] for h in heads], 1)
        ks = np.concatenate([W_qkv[:, 192 * h + 64:192 * h + 128] for h in heads], 1)
        vs = np.concatenate([W_qkv[:, 192 * h + 128:192 * h + 192] for h in heads], 1)
        bq = np.concatenate([b_qkv[192 * h:192 * h + 64] for h in heads])
        bk = np.concatenate([b_qkv[192 * h + 64:192 * h + 128] for h in heads])
        in_maps.append({
            "xT": np.ascontiguousarray(x[b].T).astype(bf),
            "wq": np.ascontiguousarray(qs).astype(bf),
            "wk": np.ascontiguousarray(ks).astype(bf),
            "wv": np.ascontiguousarray(vs).astype(bf),
            "wo": np.ascontiguousarray(W_out[CH * g:CH * (g + 1)]).astype(bf),
            "bqk": np.stack([bq[:128], bq[128:], bk[:128], bk[128:]])
                     .reshape(4, 128, 1).astype(np.float32),
        })
    return in_maps


_PROGRAM_CACHE = {}


def _get_program(S):
    if S not in _PROGRAM_CACHE:
        _PROGRAM_CACHE[S] = build_core_program(S=S)
    return _PROGRAM_CACHE[S]


class PjrtRunner:
    """Reusable compiled SPMD executable (no donation, so it can be re-run
    back-to-back on device-resident inputs for timing)."""

    def __init__(self, nc, n_cores=8):
        import jax
        from jax.sharding import Mesh, PartitionSpec
        from jax.experimental.shard_map import shard_map
        from concourse import bass2jax, mybir as mb

        bass2jax.install_neuronx_cc_hook()
        self.nc = nc
        self.n_cores = n_cores
        in_names, out_names, out_avals, zero_outs = [], [], [], []
        for alloc in nc.m.functions[0].allocations:
            if not isinstance(alloc, mb.MemoryLocationSet):
                continue
            name = alloc.memorylocations[0].name
            if alloc.kind == "ExternalInput":
                in_names.append(name)
            elif alloc.kind == "ExternalOutput":
                out_names.append(name)
                shape = tuple(alloc.tensor_shape)
                dtype = mb.dt.np(alloc.dtype)
                out_avals.append(jax.core.ShapedArray(shape, dtype))
                zero_outs.append(np.zeros(shape, dtype))
        self.in_names = list(in_names)
        self.out_names = out_names
        self.out_avals = out_avals
        self.zero_outs = zero_outs
        n_params = len(in_names)
        all_names = in_names + out_names

        def _body(*args):
            outs = bass2jax._bass_exec_p.bind(
                *args,
                out_avals=tuple(out_avals),
                in_names=tuple(all_names),
                out_names=tuple(out_names),
                lowering_input_output_aliases=(),
                sim_require_finite=True,
                sim_require_nnan=True,
                nc=nc,
            )
            return tuple(outs)

        devices = jax.devices()[:n_cores]
        self.mesh = Mesh(np.asarray(devices), ("core",))
        in_specs = (PartitionSpec("core"),) * (n_params + len(out_names))
        out_specs = (PartitionSpec("core"),) * len(out_names)
        self.fn = jax.jit(
            shard_map(_body, mesh=self.mesh, in_specs=in_specs,
                      out_specs=out_specs, check_rep=False),
            keep_unused=True,
        )
        self._dev_args = None

    def stage(self, in_maps):
        """Concatenate per-core inputs, upload once, keep device arrays."""
        import jax
        from jax.sharding import NamedSharding, PartitionSpec
        n = self.n_cores
        concat = [
            np.concatenate([np.asarray(in_maps[c][k]) for c in range(n)], axis=0)
            for k in self.in_names
        ]
        concat += [
            np.zeros((n * z.shape[0], *z.shape[1:]), z.dtype)
            for z in self.zero_outs
        ]
        sh = NamedSharding(self.mesh, PartitionSpec("core"))
        self._dev_args = [jax.device_put(a, sh) for a in concat]

    def run(self):
        outs = self.fn(*self._dev_args)
        # keep device arrays for reuse; pull results to host
        res = []
        for c in range(self.n_cores):
            res.append({
                name: np.asarray(outs[i]).reshape(
                    self.n_cores, *self.out_avals[i].shape)[c]
                for i, name in enumerate(self.out_names)
            })
        return res

    def time_iters(self, iters=20):
        import time
        import jax
        outs = self.fn(*self._dev_args)
        jax.block_until_ready(outs)
        t0 = time.perf_counter()
        for _ in range(iters):
            outs = self.fn(*self._dev_args)
        jax.block_until_ready(outs)
        t1 = time.perf_counter()
        return (t1 - t0) / iters


_RUNNER_CACHE = {}


def get_runner(S):
    if S not in _RUNNER_CACHE:
        _RUNNER_CACHE[S] = PjrtRunner(_get_program(S))
    return _RUNNER_CACHE[S]


def combine_outputs(results, W_qkv, b_qkv, W_out, b_out, B, S, D):
    b_v = np.concatenate([b_qkv[192 * h + 128:192 * h + 192] for h in range(16)])
    corr = (b_v.astype(np.float64) @ W_out.astype(np.float64)).astype(np.float32)
    corr += b_out
    out = np.zeros((B, S, D), np.float32)
    for c in range(8):
        out[c // 4] += results[c]["out"].astype(np.float32)
    out += corr[None, None, :]
    return out


def kernel(x, W_qkv, b_qkv, W_out, b_out):
    x = np.asarray(x)
    W_qkv = np.asarray(W_qkv)
    b_qkv = np.asarray(b_qkv)
    W_out = np.asarray(W_out)
    b_out = np.asarray(b_out)
    B, S, D = x.shape

    runner = get_runner(S)
    runner.stage(make_in_maps(x, W_qkv, b_qkv, W_out))
    results = runner.run()
    return combine_outputs(results, W_qkv, b_qkv, W_out, b_out, B, S, D)
